# revision 17
# baseline (speedup 1.0000x reference)
"""Bidirectional LSTM Trainium2 Bass kernel.

Problem: T=128, B=128, IN=512, H=512, OUT=512 (fp32 reference).
Sharding: data-parallel over batch + direction-parallel:
  cores 0-3: forward LSTM, batch slices 0:32, 32:64, 64:96, 96:128
  cores 4-7: backward LSTM (time-reversed x), same batch slices
Each core (phases overlap via dependency scheduling):
  phase 1: xw[t] = x[t] @ W_ih.T + (b_ih + b_hh), 4 timesteps per GEMM
           (M=128), bias added via a rank-1 (K=1) matmul, result kept in
           an 8-chunk SBUF ring consumed by phase 2
  phase 2: 128 sequential LSTM steps:
           gates = xw[t] (seeded into PSUM via a column-selection matmul
           against ident128, which also sets PSUM has_written)
                 + h[t-1] @ W_hh.T (4 K-tile matmuls per 512-col bank)
           sigmoid/tanh on ScalarE, cell update on VectorE,
           h transposed on TensorE for the next step's stationary operand
  phase 3: partial out[t] = h[t] @ W_lin[:, dir*H:(dir+1)*H].T into an
           SBUF buffer, one final DMA to DRAM
Host combines: out = out_fwd + flip_t(out_bwd) + b_lin.

All matmuls run in bf16 (fp32 PSUM accumulation); the cell state c stays
fp32. Gate columns are host-permuted to [o f i g] per 256-wide half so
one sigmoid instruction covers o,f,i contiguously.
"""

import sys

sys.path.insert(0, "/opt/trn_rl_repo")

import functools

import ml_dtypes
import numpy as np

import concourse.bass as bass
import concourse.tile as tile
from concourse import bacc, mybir
from concourse.bass_utils import run_bass_kernel_spmd

T, B, IN, H, OUT = 128, 128, 512, 512, 512
NCORES = 8
BL = B // 4  # batch per core (4 cores per direction)
G4 = 4 * H  # 2048 gate columns
KT = IN // 128  # 4 K-tiles of 128
NB = G4 // 512  # 4 psum banks of 512 gate cols
TCH = T // 4  # 32 chunks of 4 timesteps for phase 1/3
RING = 8  # xw ring depth (chunks)

BF16 = mybir.dt.bfloat16
FP32 = mybir.dt.float32
AF = mybir.ActivationFunctionType


def build_nc():
    nc = bacc.Bacc(None, target_bir_lowering=False)
    xT = nc.dram_tensor("xT", [128, TCH, KT, 4, BL], BF16, kind="ExternalInput")
    wih = nc.dram_tensor("wih", [128, KT, G4], BF16, kind="ExternalInput")
    whh = nc.dram_tensor("whh", [128, KT, G4], BF16, kind="ExternalInput")
    bias = nc.dram_tensor("biasr", [1, G4], BF16, kind="ExternalInput")
    ones = nc.dram_tensor("ones", [1, 128], BF16, kind="ExternalInput")
    wlin = nc.dram_tensor("wlin", [128, KT, OUT], BF16, kind="ExternalInput")
    id32 = nc.dram_tensor("id32", [BL, BL], BF16, kind="ExternalInput")
    id128 = nc.dram_tensor("id128", [128, 128], BF16, kind="ExternalInput")
    outp = nc.dram_tensor("outp", [128, TCH, OUT], FP32, kind="ExternalOutput")

    with tile.TileContext(nc) as tc:
        with (
            tc.tile_pool(name="const", bufs=1) as constp,
            tc.tile_pool(name="xwring", bufs=RING) as ringp,
            tc.tile_pool(name="p1x", bufs=4) as p1x,
            tc.tile_pool(name="acts", bufs=2) as acts_p,
            tc.tile_pool(name="tmps", bufs=2) as tmps_p,
            tc.tile_pool(name="p1ps", bufs=1, space="PSUM") as p1ps,
            tc.tile_pool(name="ps2", bufs=1, space="PSUM") as ps2,
            tc.tile_pool(name="psT", bufs=1, space="PSUM") as psT,
            tc.tile_pool(name="ps3", bufs=1, space="PSUM") as ps3,
        ):
            id32_sb = constp.tile([BL, BL], BF16)
            nc.sync.dma_start(id32_sb[:], id32[:])
            id128_sb = constp.tile([128, 128], BF16)
            nc.sync.dma_start(id128_sb[:], id128[:])
            wih_sb = constp.tile([128, KT, G4], BF16)
            nc.sync.dma_start(wih_sb[:], wih[:])
            whh_sb = constp.tile([128, KT, G4], BF16)
            nc.sync.dma_start(whh_sb[:], whh[:])
            bias_sb = constp.tile([1, G4], BF16)
            nc.sync.dma_start(bias_sb[:], bias[:])
            ones_sb = constp.tile([1, 128], BF16)
            nc.sync.dma_start(ones_sb[:], ones[:])
            wlin_sb = constp.tile([128, KT, OUT], BF16)
            nc.sync.dma_start(wlin_sb[:], wlin[:])
            # h^T history: slot (t//4, k, t%4) holds h_t[128k:128k+128, :].
            # Chunk-major so phase 3 reads [128, 4, 32] contiguous per k.
            hT_all = constp.tile([128, TCH, KT, 4, BL], BF16)
            hT0 = constp.tile([128, KT, BL], BF16)
            nc.vector.memset(hT0[:], 0.0)
            c_st = constp.tile([BL, H], FP32)
            nc.vector.memset(c_st[:], 0.0)
            out_all = constp.tile([128, TCH, OUT], FP32)

            # ---- phase 1: xw = x @ W_ih.T + bias, 4 timesteps per GEMM ----
            xw_tiles = []
            for ch in range(TCH):
                xt = p1x.tile([128, KT, 4, BL], BF16, tag="xt")
                nc.sync.dma_start(xt[:], xT[:, ch])
                xwr = ringp.tile([128, G4], BF16, tag="xw")
                xw_tiles.append(xwr)
                for half in range(2):
                    pxw = p1ps.tile([128, 1024], FP32, tag="pxw")
                    for nb2 in range(2):
                        cp = slice(512 * nb2, 512 * nb2 + 512)
                        cg = slice(
                            1024 * half + 512 * nb2, 1024 * half + 512 * nb2 + 512
                        )
                        for k in range(KT):
                            nc.tensor.matmul(
                                pxw[:, cp],
                                xt[:, k],
                                wih_sb[:, k, cg],
                                start=(k == 0),
                                stop=False,
                            )
                        # bias via rank-1 matmul: ones.T @ bias_row
                        nc.tensor.matmul(
                            pxw[:, cp],
                            ones_sb[:],
                            bias_sb[:, cg],
                            start=False,
                            stop=True,
                        )
                    nc.vector.tensor_copy(
                        xwr[:, 1024 * half : 1024 * half + 1024], pxw[:]
                    )

            # ---- phase 2: the recurrence ----
            for t in range(T):
                ch, ti = t // 4, t % 4
                gates = ps2.tile([BL, G4], FP32, tag="gates")
                # seed PSUM with xw row-block ti (sets has_written)
                for nb in range(NB):
                    cs = slice(512 * nb, 512 * nb + 512)
                    nc.tensor.matmul(
                        gates[:, cs],
                        id128_sb[:, 32 * ti : 32 * ti + 32],
                        xw_tiles[ch][:, cs],
                        start=True,
                        stop=False,
                    )
                # accumulate h_{t-1} @ W_hh.T
                for nb in range(NB):
                    cs = slice(512 * nb, 512 * nb + 512)
                    for k in range(KT):
                        hT_prev = (
                            hT0[:, k]
                            if t == 0
                            else hT_all[:, (t - 1) // 4, k, (t - 1) % 4, :]
                        )
                        nc.tensor.matmul(
                            gates[:, cs],
                            hT_prev,
                            whh_sb[:, k, cs],
                            start=False,
                            stop=(k == KT - 1),
                        )
                acts = acts_p.tile([BL, G4], FP32, tag="acts")
                tct = tmps_p.tile([BL, H], FP32, tag="tct")
                fc = tmps_p.tile([BL, H], FP32, tag="fc")
                ig = tmps_p.tile([BL, H], FP32, tag="ig")
                h_sb = tmps_p.tile([BL, H], BF16, tag="hsb")
                # per 1024-col half: [o f i g] x 256; hidden units 256q..
                for q in range(2):
                    b0 = 1024 * q
                    hs = slice(256 * q, 256 * q + 256)
                    nc.scalar.activation(
                        acts[:, b0 : b0 + 768], gates[:, b0 : b0 + 768], AF.Sigmoid
                    )
                    nc.scalar.activation(
                        acts[:, b0 + 768 : b0 + 1024],
                        gates[:, b0 + 768 : b0 + 1024],
                        AF.Tanh,
                    )
                    o_ = acts[:, b0 : b0 + 256]
                    f_ = acts[:, b0 + 256 : b0 + 512]
                    i_ = acts[:, b0 + 512 : b0 + 768]
                    g_ = acts[:, b0 + 768 : b0 + 1024]
                    nc.vector.tensor_mul(fc[:, hs], f_, c_st[:, hs])
                    nc.vector.tensor_mul(ig[:, hs], i_, g_)
                    nc.vector.tensor_add(c_st[:, hs], fc[:, hs], ig[:, hs])
                    nc.scalar.activation(tct[:, hs], c_st[:, hs], AF.Tanh)
                    nc.vector.tensor_mul(h_sb[:, hs], o_, tct[:, hs])
                hTp = psT.tile([128, KT, BL], BF16, tag="hTp")
                for k in range(KT):
                    nc.tensor.transpose(
                        hTp[:, k], h_sb[:, 128 * k : 128 * k + 128], id32_sb[:]
                    )
                nc.vector.tensor_copy(hT_all[:, ch, :, ti, :], hTp[:])

            # ---- phase 3: partial linear out = h @ W_lin_half.T ----
            for ch in range(TCH):
                po = ps3.tile([128, OUT], FP32, tag="po")
                for k in range(KT):
                    nc.tensor.matmul(
                        po[:],
                        hT_all[:, ch, k],
                        wlin_sb[:, k],
                        start=(k == 0),
                        stop=(k == KT - 1),
                    )
                nc.vector.tensor_copy(out_all[:, ch, :], po[:])
            nc.sync.dma_start(outp[:], out_all[:])
    nc.compile()
    return nc


@functools.lru_cache(maxsize=1)
def _program():
    return build_nc()


def _gate_perm():
    # PyTorch gate row order: i (0:H), f (H:2H), g (2H:3H), o (3H:4H).
    # Target layout per 1024-col half q: [o f i g] x 256 covering hidden
    # units 256q:256q+256, so sigmoid spans 768 contiguous cols.
    perm = []
    for q in range(2):
        perm += list(range(3 * H + 256 * q, 3 * H + 256 * q + 256))  # o
        perm += list(range(1 * H + 256 * q, 1 * H + 256 * q + 256))  # f
        perm += list(range(0 * H + 256 * q, 0 * H + 256 * q + 256))  # i
        perm += list(range(2 * H + 256 * q, 2 * H + 256 * q + 256))  # g
    return np.asarray(perm)


def _prep_core(x, W_ih, W_hh, b_ih, b_hh, W_lin, direction, bs):
    perm = _gate_perm()
    bf16 = ml_dtypes.bfloat16
    xs = np.asarray(x)[:, bs : bs + BL, :]
    if direction == 1:
        xs = xs[::-1]
    # xT[p, ch, k, ti, b] = xs[4*ch+ti, b, 128k+p]
    xT = np.ascontiguousarray(
        xs.reshape(TCH, 4, BL, KT, 128).transpose(4, 0, 3, 1, 2)
    ).astype(bf16)
    Wp_ih = np.asarray(W_ih)[perm]  # [G4, IN]
    Wp_hh = np.asarray(W_hh)[perm]
    bp = (np.asarray(b_ih) + np.asarray(b_hh))[perm].astype(np.float32)
    wih = np.ascontiguousarray(Wp_ih.T.reshape(KT, 128, G4).transpose(1, 0, 2)).astype(
        bf16
    )
    whh = np.ascontiguousarray(Wp_hh.T.reshape(KT, 128, G4).transpose(1, 0, 2)).astype(
        bf16
    )
    biasr = bp.reshape(1, G4).astype(bf16)
    ones = np.ones((1, 128), dtype=bf16)
    Wl = np.asarray(W_lin)[:, direction * H : (direction + 1) * H]  # [OUT, H]
    wlin = np.ascontiguousarray(Wl.T.reshape(KT, 128, OUT).transpose(1, 0, 2)).astype(
        bf16
    )
    return {
        "xT": xT,
        "wih": wih,
        "whh": whh,
        "biasr": biasr,
        "ones": ones,
        "wlin": wlin,
        "id32": np.eye(BL, dtype=bf16),
        "id128": np.eye(128, dtype=bf16),
    }


def run_cores(inputs, trace=False):
    """Build per-core in_maps, run on 8 cores, return BassKernelResults."""
    in_maps = []
    for core in range(NCORES):
        direction = core // 4
        bs = (core % 4) * BL
        wk = "f" if direction == 0 else "b"
        in_maps.append(
            _prep_core(
                inputs["x"],
                inputs[f"W_ih_{wk}"],
                inputs[f"W_hh_{wk}"],
                inputs[f"b_ih_{wk}"],
                inputs[f"b_hh_{wk}"],
                inputs["W_lin"],
                direction,
                bs,
            )
        )
    nc = _program()
    return run_bass_kernel_spmd(nc, in_maps, list(range(NCORES)), trace=trace)


def _assemble(results, b_lin):
    # per-core outp: [128(=4 ti x 32 b), TCH, OUT] in compute-time order
    out = np.zeros((T, B, OUT), np.float32)
    for core in range(NCORES):
        direction = core // 4
        bs = (core % 4) * BL
        dev = np.asarray(results[core]["outp"], np.float32)  # [128, TCH, OUT]
        # t = 4*ch + ti, partition p = 32*ti + b
        part = (
            dev.reshape(4, BL, TCH, OUT).transpose(2, 0, 1, 3).reshape(T, BL, OUT)
        )
        if direction == 1:
            part = part[::-1]
        out[:, bs : bs + BL, :] += part
    out += np.asarray(b_lin, np.float32)[None, None, :]
    return out


def kernel(**inputs):
    res = run_cores(inputs, trace=False)
    return _assemble(res.results, inputs["b_lin"])
